# revision 7
# baseline (speedup 1.0000x reference)
"""ConvLSTM Trainium2 kernel (8 NeuronCores, data-parallel over batch).

Math (per timestep t, batched over B):
  att  = softmax(tanh(mean_s(x) @ fc1.T + b1) @ fc2.T + b2)          [B, C]
  y    = conv3d(x * att) + conv_b  -> flatten                         [B, 1728]
  gates= y @ w_ih.T + b_ih + h @ w_hh.T + b_hh                        [B, 256]
  LSTM cell -> h_t; out = mean_t(h_t) @ fc_w.T + fc_b                 [B, 3]

Strategy (vs the per-timestep serial scan, which is latency-bound at
~2.1us/step through 5 cross-engine hops x 128 steps):

  * conv3d on the 3x3x3 grid is a linear map; fold into the LSTM input
    projection on host: W_big = w_ih @ W_conv [256, 1728]. The feedforward
    batches over all B*T tokens in bf16 (4x PE throughput vs fp32).
  * The recurrence is solved by PICARD ITERATION over the whole sequence:
    given h^{k-1} for all t, gates = gx + whh.T @ h^{k-1}_{t-1} is one
    batched matmul; c^k follows from ONE DVE tensor_tensor_scan instruction
    (state = A_t * state + B_t along the free dim); h^k = sig(o) tanh(c).
    The map has contraction ratio ~0.18 (w_hh ~ N(0, 0.05^2)), so K=5
    iterations reach the bf16 noise floor (~6e-3 output rel err; measured
    offline against the fp64 recurrence).
  * Tokens are b-major (tok = b*T + t) so one scan spans all batches of a
    block; A is forced to 0 at each t=0 column, which resets the scan state
    at batch boundaries exactly.
  * All gate nonlinearities use tanh only (sig(z) = 0.5 tanh(z/2) + 0.5,
    affine folded into DVE ops; cell carries S=2c, H2=2h with 0.5 folded
    into whh and fc_w) so the ACT engine stays on the {tanh, exp, identity}
    table set -- no table reloads.

Sharding: batch 128 -> 16 per core; 4 blocks of 4 batches pipeline through
DMA/PE/ACT/DVE.
"""

import os
import numpy as np
import ml_dtypes
from contextlib import ExitStack

import concourse.bass as bass
import concourse.tile as tile
import concourse.mybir as mybir
from concourse import bacc
from concourse.bass_utils import run_bass_kernel_spmd
from concourse.masks import make_identity

FP32 = mybir.dt.float32
BF16 = mybir.dt.bfloat16
BF = ml_dtypes.bfloat16
AL = mybir.AluOpType

B, T, C = 128, 128, 64
HID = 64
S3 = 27                    # 3*3*3 spatial positions
KIN = C * S3               # 1728
NCH = 14                   # contraction chunks of 128 (padded)
KPAD = NCH * 128           # 1792
NCORES = 8
BL = B // NCORES           # 16 batch per core
NTOK = BL * T              # 2048 tokens per core, tok = b*T + t
NBLK = 4
BB = BL // NBLK            # 4 batches per block
BLKTOK = BB * T            # 512 tokens per block
KITER = int(os.environ.get("KITER", "5"))

_CACHE = {}


# ---------------------------------------------------------------- host folds
def _conv_matrix(conv_w):
    """[HID, C, 3, 3, 3] -> dense [HID*27, C*27] linear map of the same-padded
    3x3x3 conv on a 3x3x3 grid."""
    pos = np.arange(S3)
    pz, py, px = pos // 9, (pos // 3) % 3, pos % 3
    rows = np.arange(HID) * S3
    cols = np.arange(C) * S3
    Wc = np.zeros((HID * S3, C * S3), np.float64)
    for p in range(S3):
        for q in range(S3):
            kz = pz[q] - pz[p] + 1
            ky = py[q] - py[p] + 1
            kx = px[q] - px[p] + 1
            if 0 <= kz < 3 and 0 <= ky < 3 and 0 <= kx < 3:
                Wc[np.ix_(rows + p, cols + q)] = conv_w[:, :, kz, ky, kx]
    return Wc


def _fold_weights(fc1_w, fc1_b, fc2_w, fc2_b, conv_w, conv_b,
                  w_ih, w_hh, b_ih, b_hh, fc_w, fc_b):
    w_ih = np.asarray(w_ih, np.float64)
    Wc = _conv_matrix(np.asarray(conv_w, np.float64))
    W_big = w_ih @ Wc                                           # [256, 1728]
    b_all = (w_ih @ np.repeat(np.asarray(conv_b, np.float64), S3)
             + np.asarray(b_ih, np.float64) + np.asarray(b_hh, np.float64))

    # gate row order (g, o, i, f); prescale rows so ONE tanh covers all:
    #   G = tanh(g), O = tanh(o/2), I = tanh(i/2), F = tanh(f/2)
    # Layout puts the stt operand pairs (I,G) and (O,TC) on EQUAL base
    # partitions (a BIR constraint for two-SBUF-input DVE ops):
    #   slot0 = [g; o], slot1 = [i; f]
    H = HID
    order = np.concatenate([np.arange(2*H, 3*H), np.arange(3*H, 4*H),
                            np.arange(0, H), np.arange(H, 2*H)])
    s = np.concatenate([np.full(H, 1.0), np.full(H, 0.5),
                        np.full(H, 0.5), np.full(H, 0.5)])
    W_eff = W_big[order] * s[:, None]
    b_eff = b_all[order] * s
    # whh consumes H2 = 2h -> fold 0.5
    whh_eff = np.asarray(w_hh, np.float64)[order] * s[:, None] * 0.5

    WbT = np.zeros((KPAD, 256), np.float32)
    WbT[:KIN] = W_eff.T.astype(np.float32)
    WbT = np.ascontiguousarray(WbT.reshape(NCH, 128, 256)).astype(BF)

    # channel-membership matrices (0/1; the 1/27 mean factor folds into fc1)
    k = np.arange(KPAD)
    cid = np.where(k < KIN, k // S3, -1)
    Em = (cid[:, None] == np.arange(C)[None, :]).astype(np.float32)
    Em = np.ascontiguousarray(Em.reshape(NCH, 128, C)).astype(BF)
    Eb = (np.arange(C)[:, None] == cid[None, :]).astype(np.float32)
    Eb = np.ascontiguousarray(Eb.reshape(C, NCH, 128).transpose(1, 0, 2)).astype(BF)

    return {
        "wbigT": WbT,
        "em": Em,
        "eb": Eb,
        "fc1wT": np.ascontiguousarray(np.asarray(fc1_w, np.float64).T / S3).astype(BF),
        "fc1b": np.asarray(fc1_b, np.float32).reshape(C, 1),
        "fc2wT": np.ascontiguousarray(np.asarray(fc2_w, np.float32).T).astype(BF),
        "fc2b": np.asarray(fc2_b, np.float32).reshape(C, 1),
        "whhT": np.ascontiguousarray(whh_eff.T).astype(BF),        # [64, 256]
        "bh0": np.ascontiguousarray(b_eff[:128].reshape(128, 1)).astype(np.float32),
        "bh1": np.ascontiguousarray(b_eff[128:].reshape(128, 1)).astype(np.float32),
        "fcwT": np.ascontiguousarray(np.asarray(fc_w, np.float64).T * (0.5 / T)).astype(BF),
        "fcb": np.asarray(fc_b, np.float32).reshape(3, 1),
    }


def _shard_x(x):
    """x [B, T, C, 3,3,3] -> per-core [NCH, 128, NTOK] bf16, tok = b*T + t."""
    x = np.asarray(x, np.float32).reshape(B, T, KIN).astype(BF)
    shards = []
    for c in range(NCORES):
        xc = x[c * BL:(c + 1) * BL]                      # [16, T, 1728]
        xt = np.ascontiguousarray(xc.transpose(2, 0, 1)).reshape(KIN, NTOK)
        xp = np.zeros((KPAD, NTOK), BF)
        xp[:KIN] = xt
        shards.append(np.ascontiguousarray(xp.reshape(NCH, 128, NTOK)))
    return shards


# ---------------------------------------------------------------- device build
def _build():
    nc = bacc.Bacc("TRN2", target_bir_lowering=False)
    d_x = nc.dram_tensor("xT", [NCH, 128, NTOK], BF16, kind="ExternalInput")
    d_wbig = nc.dram_tensor("wbigT", [NCH, 128, 256], BF16, kind="ExternalInput")
    d_em = nc.dram_tensor("em", [NCH, 128, C], BF16, kind="ExternalInput")
    d_eb = nc.dram_tensor("eb", [NCH, C, 128], BF16, kind="ExternalInput")
    d_fc1w = nc.dram_tensor("fc1wT", [C, C], BF16, kind="ExternalInput")
    d_fc1b = nc.dram_tensor("fc1b", [C, 1], FP32, kind="ExternalInput")
    d_fc2w = nc.dram_tensor("fc2wT", [C, C], BF16, kind="ExternalInput")
    d_fc2b = nc.dram_tensor("fc2b", [C, 1], FP32, kind="ExternalInput")
    d_whh = nc.dram_tensor("whhT", [HID, 256], BF16, kind="ExternalInput")
    d_bh0 = nc.dram_tensor("bh0", [128, 1], FP32, kind="ExternalInput")
    d_bh1 = nc.dram_tensor("bh1", [128, 1], FP32, kind="ExternalInput")
    d_fcw = nc.dram_tensor("fcwT", [HID, 3], BF16, kind="ExternalInput")
    d_fcb = nc.dram_tensor("fcb", [3, 1], FP32, kind="ExternalInput")
    d_out = nc.dram_tensor("out", [3, BL], FP32, kind="ExternalOutput")

    TANH = mybir.ActivationFunctionType.Tanh
    EXP = mybir.ActivationFunctionType.Exp
    IDENT = mybir.ActivationFunctionType.Identity

    with tile.TileContext(nc) as tc, ExitStack() as ctx:
        consts = ctx.enter_context(tc.tile_pool(name="consts", bufs=1))
        xpool = ctx.enter_context(tc.tile_pool(name="x", bufs=2))
        xapool = ctx.enter_context(tc.tile_pool(name="xa", bufs=2))
        gxpool = ctx.enter_context(tc.tile_pool(name="gx", bufs=NBLK))
        small = ctx.enter_context(tc.tile_pool(name="small", bufs=3))
        tpool = ctx.enter_context(tc.tile_pool(name="t", bufs=2))
        spool = ctx.enter_context(tc.tile_pool(name="s", bufs=3))
        ps_stat = ctx.enter_context(tc.tile_pool(name="ps_stat", bufs=2, space="PSUM"))
        ps_ab = ctx.enter_context(tc.tile_pool(name="ps_ab", bufs=2, space="PSUM"))
        ps_g = ctx.enter_context(tc.tile_pool(name="ps_g", bufs=2, space="PSUM"))
        ps_g2 = ctx.enter_context(tc.tile_pool(name="ps_g2", bufs=1, space="PSUM"))

        # ---- constants
        wbig = consts.tile([128, NCH, 256], BF16)
        nc.sync.dma_start(wbig[:], d_wbig.ap().rearrange("c p f -> p c f"))
        em = consts.tile([128, NCH, C], BF16)
        nc.sync.dma_start(em[:], d_em.ap().rearrange("c p f -> p c f"))
        eb = consts.tile([C, NCH, 128], BF16)
        nc.sync.dma_start(eb[:], d_eb.ap().rearrange("c p f -> p c f"))
        fc1w = consts.tile([C, C], BF16); nc.sync.dma_start(fc1w[:], d_fc1w.ap())
        fc1b = consts.tile([C, 1], FP32); nc.sync.dma_start(fc1b[:], d_fc1b.ap())
        fc2w = consts.tile([C, C], BF16); nc.sync.dma_start(fc2w[:], d_fc2w.ap())
        fc2b = consts.tile([C, 1], FP32); nc.sync.dma_start(fc2b[:], d_fc2b.ap())
        whh = consts.tile([HID, 256], BF16); nc.sync.dma_start(whh[:], d_whh.ap())
        bh0 = consts.tile([128, 1], FP32); nc.sync.dma_start(bh0[:], d_bh0.ap())
        bh1 = consts.tile([128, 1], FP32); nc.sync.dma_start(bh1[:], d_bh1.ap())
        fcw = consts.tile([HID, 3], BF16); nc.sync.dma_start(fcw[:], d_fcw.ap())
        fcb = consts.tile([3, 1], FP32); nc.sync.dma_start(fcb[:], d_fcb.ap())
        id128 = consts.tile([128, 128], BF16); make_identity(nc, id128[:])
        ones_col = consts.tile([C, 1], BF16); nc.gpsimd.memset(ones_col[:], 1.0)
        ones_row = consts.tile([1, C], BF16); nc.gpsimd.memset(ones_row[:], 1.0)
        # H2 = 2h history, [64, b, t+1]; column t=0 stays zero
        hbuf = consts.tile([HID, BL, T + 1], BF16)
        nc.vector.memset(hbuf[:, :, 0:1], 0.0)

        for blk in range(NBLK):
            n0 = blk * BLKTOK
            b0 = blk * BB
            # -------- feedforward: attention + big matmul -> gx ------------
            x_blk = xpool.tile([128, NCH, BLKTOK], BF16, tag="x")
            for ch in range(NCH):
                nc.sync.dma_start(x_blk[:, ch, :], d_x.ap()[ch, :, n0:n0 + BLKTOK])

            # channel sums: accumulate Em.T @ x over chunks -> [C, 512]
            xsum_ps = ps_stat.tile([C, BLKTOK], FP32, tag="stat")
            for ch in range(NCH):
                nc.tensor.matmul(xsum_ps[:], em[:, ch, :], x_blk[:, ch, :],
                                 start=(ch == 0), stop=(ch == NCH - 1))
            xsum = small.tile([C, BLKTOK], BF16, tag="xsum")
            nc.scalar.copy(xsum[:], xsum_ps[:])

            # attention MLP (all [C, 512], c on partitions)
            a_ps = ps_stat.tile([C, BLKTOK], FP32, tag="stat")
            nc.tensor.matmul(a_ps[:], fc1w[:], xsum[:], start=True, stop=True)
            a_sb = small.tile([C, BLKTOK], BF16, tag="a")
            nc.scalar.activation(a_sb[:], a_ps[:], TANH, bias=fc1b[:])
            l_ps = ps_stat.tile([C, BLKTOK], FP32, tag="stat")
            nc.tensor.matmul(l_ps[:], fc2w[:], a_sb[:], start=True, stop=True)
            e_sb = small.tile([C, BLKTOK], BF16, tag="e")
            nc.scalar.activation(e_sb[:], l_ps[:], EXP, bias=fc2b[:])

            # softmax: denominator via ones-matmul, reciprocal, broadcast back
            ssum_ps = ps_stat.tile([1, BLKTOK], FP32, tag="stat")
            nc.tensor.matmul(ssum_ps[:], ones_col[:], e_sb[:], start=True, stop=True)
            rin = small.tile([1, BLKTOK], BF16, tag="rin")
            with nc.allow_low_precision(reason="softmax denom fits bf16"):
                nc.vector.reciprocal(rin[:], ssum_ps[:])
            rb_ps = ps_stat.tile([C, BLKTOK], FP32, tag="stat")
            nc.tensor.matmul(rb_ps[:], ones_row[:], rin[:], start=True, stop=True)
            att = small.tile([C, BLKTOK], BF16, tag="att")
            nc.vector.tensor_mul(att[:], e_sb[:], rb_ps[:])

            # x * att (broadcast over the 27 spatial positions via Eb matmul)
            xa_blk = xapool.tile([128, NCH, BLKTOK], BF16, tag="xa")
            for ch in range(NCH):
                ab_ps = ps_ab.tile([128, BLKTOK], FP32, tag="ab")
                nc.tensor.matmul(ab_ps[:], eb[:, ch, :], att[:],
                                 start=True, stop=True)
                nc.vector.tensor_mul(xa_blk[:, ch, :], x_blk[:, ch, :], ab_ps[:])

            # big matmul: gates_x = W_big @ x_a; keep PSUM halves for iter 1
            g_half = []
            gx_blk = gxpool.tile([128, 2, BLKTOK], BF16, tag="gx")
            for half in range(2):
                g_ps = ps_g.tile([128, BLKTOK], FP32, tag="g")
                for ch in range(NCH):
                    nc.tensor.matmul(
                        g_ps[:], wbig[:, ch, half * 128:(half + 1) * 128],
                        xa_blk[:, ch, :],
                        start=(ch == 0), stop=(ch == NCH - 1))
                nc.scalar.activation(gx_blk[:, half, :], g_ps[:], IDENT,
                                     bias=(bh0[:] if half == 0 else bh1[:]))
                g_half.append(g_ps)

            # -------- Picard iterations over this block's 4 batches --------
            for k in range(KITER):
                tt = tpool.tile([128, 2, BLKTOK], BF16, tag="tt")
                if k == 0:
                    # h^0 = 0: tanh directly on the feedforward PSUM (+bias)
                    nc.scalar.activation(tt[:, 0, :], g_half[0][:], TANH,
                                         bias=bh0[:])
                    nc.scalar.activation(tt[:, 1, :], g_half[1][:], TANH,
                                         bias=bh1[:])
                else:
                    g2 = ps_g2.tile([128, 2, BLKTOK], FP32, tag="g2")
                    hs = hbuf[:, b0:b0 + BB, 0:T]
                    for half in range(2):
                        nc.tensor.matmul(g2[:, half, :], id128[:],
                                         gx_blk[:, half, :],
                                         start=True, stop=True)
                        nc.tensor.matmul(g2[:, half, :],
                                         whh[:, half * 128:(half + 1) * 128],
                                         hs, start=False, stop=False,
                                         skip_group_check=True)
                    nc.scalar.activation(tt[:], g2[:], TANH)
                # A = sig(f) = 0.5*F + 0.5, zeroed at t=0 of each batch
                A = spool.tile([HID, BB, T], BF16, tag="A")
                nc.vector.tensor_scalar(A[:], tt[64:128, 1, :].rearrange(
                    "p (b t) -> p b t", b=BB), 0.5, 0.5, AL.mult, AL.add)
                nc.vector.memset(A[:, :, 0:1], 0.0)
                # Bv = (I+1)*G = 2 sig(i) tanh(g)  (I and G share base partition)
                Bv = spool.tile([HID, BLKTOK], BF16, tag="Bv")
                nc.vector.scalar_tensor_tensor(Bv[:], tt[0:64, 1, :], 1.0,
                                               tt[0:64, 0, :], AL.add, AL.mult)
                # S = 2c via one scan: state = A*state + Bv
                S = spool.tile([HID, BLKTOK], BF16, tag="S")
                nc.vector.tensor_tensor_scan(
                    S[:], A[:].rearrange("p b t -> p (b t)"), Bv[:], 0.0,
                    AL.mult, AL.add)
                # TC = tanh(c), placed on partitions 64:128 to pair with O
                TC = spool.tile([128, BLKTOK], BF16, tag="TC")
                nc.scalar.activation(TC[64:128, :], S[:], TANH, scale=0.5)
                # H2 = (O+1)*TC = 2h -> write shifted into hbuf
                nc.vector.scalar_tensor_tensor(
                    hbuf[:, b0:b0 + BB, 1:T + 1],
                    tt[64:128, 0, :].rearrange("p (b t) -> p b t", b=BB), 1.0,
                    TC[64:128, :].rearrange("p (b t) -> p b t", b=BB),
                    AL.add, AL.mult)

        # ================= head ============================================
        hsum = small.tile([HID, BL], BF16, tag="hsum")
        with nc.allow_low_precision(reason="h mean fits bf16"):
            nc.vector.tensor_reduce(out=hsum[:], in_=hbuf[:, :, 1:T + 1],
                                    op=AL.add, axis=mybir.AxisListType.X)
        o_ps = ps_stat.tile([3, BL], FP32, tag="stat")
        nc.tensor.matmul(o_ps[:], fcw[:], hsum[:], start=True, stop=True)
        o_sb = small.tile([3, BL], FP32, tag="osb")
        nc.scalar.activation(o_sb[:], o_ps[:], IDENT, bias=fcb[:])
        nc.sync.dma_start(d_out.ap(), o_sb[:])

    nc.compile()
    return nc


def _get_nc():
    if "nc" not in _CACHE:
        _CACHE["nc"] = _build()
    return _CACHE["nc"]


# ---------------------------------------------------------------- entry point
def kernel(x, fc1_w, fc1_b, fc2_w, fc2_b, conv_w, conv_b,
           w_ih, w_hh, b_ih, b_hh, fc_w, fc_b, _trace=False, _trace_kwargs=None):
    consts = _fold_weights(fc1_w, fc1_b, fc2_w, fc2_b, conv_w, conv_b,
                           w_ih, w_hh, b_ih, b_hh, fc_w, fc_b)
    shards = _shard_x(x)
    in_maps = [dict(consts, xT=shards[c]) for c in range(NCORES)]
    nc = _get_nc()
    res = run_bass_kernel_spmd(nc, in_maps, list(range(NCORES)),
                               trace=_trace, **(_trace_kwargs or {}))
    out = np.concatenate([res.results[c]["out"].T for c in range(NCORES)], axis=0)
    if _trace:
        return out.astype(np.float32), res
    return out.astype(np.float32)
